# revision 33
# baseline (speedup 1.0000x reference)
"""EntityDisambiguationHead Trainium2 kernel (bf16 rewrite).

Computes out[b,s,e] = cos_sim(tanh(x @ W.T + b), entity_embedding[e]) for
B=4, S=128, D_in=768, D_e=256, E=100000, sharding the entity axis across
8 NeuronCores (each core handles 12544 = 98*128 entities, padded from 12500).

All wire traffic is bf16 (host casts + relayouts); GEMMs run bf16 on the PE
with f32 PSUM accumulation. Entity L2-normalization is fused into the PE
transpose by multiplying with diag(1/||e||) instead of the identity:

  per core:
    q   = tanh(x @ W.T + b)               [512, 256]   (PE, bf16)
    a   = 1/sqrt(||q_t||^2 + eps)         per token
    qnt = (q^T scaled by a)               via PE matmul vs diag(a)
    c   = 1/sqrt(||ent_e||^2 + eps)       per entity (sq+reduce+rsqrt)
    enT = ent_tile^T @ diag(c)            transposed + normalized on PE
    out = qnt^T @ enT -> bf16             [512, 12544]

Host side only shards/casts/relayouts inputs and concatenates outputs.
DMA: entity loads stream on the scalar HWDGE ring (7 big contiguous
transfers issued upfront), stores go out on the sync ring.
"""

import os
from contextlib import ExitStack

import numpy as np
from ml_dtypes import bfloat16

import concourse.bass as bass
import concourse.bacc as bacc
import concourse.mybir as mybir
import concourse.tile as tile
from concourse.masks import make_identity

F32 = mybir.dt.float32
BF16 = mybir.dt.bfloat16
AF = mybir.ActivationFunctionType
ALU = mybir.AluOpType

N_CORES = 8
E_FULL = 100000
E_PER_CORE = E_FULL // N_CORES          # 12500
E_TILES = (E_PER_CORE + 127) // 128     # 98
E_PAD = E_TILES * 128                   # 12544
T = 512                                 # tokens = 4*128
D_IN = 768
D_E = 256
EPS2 = 1e-16                            # added under sqrt ~= max(norm, 1e-8)

CHUNK = 16                              # entity tiles per load DMA


def build_nc():
    """Build the per-core Bass program (SPMD: same program on all cores)."""
    nc = bacc.Bacc("TRN2", target_bir_lowering=False, debug=False)

    # DRAM tensors in device-friendly (partition-major) layouts; host prepares.
    x_d = nc.dram_tensor("x", [128, 6 * T], BF16, kind="ExternalInput").ap()
    w_d = nc.dram_tensor("w", [128, 6 * D_E], BF16, kind="ExternalInput").ap()
    b_d = nc.dram_tensor("b", [1, D_E], BF16, kind="ExternalInput").ap()
    e_d = nc.dram_tensor("ent", [128, E_TILES * D_E], BF16, kind="ExternalInput").ap()
    o_d = nc.dram_tensor("out", [128, 4 * E_PAD], BF16, kind="ExternalOutput").ap()

    x_v = x_d.rearrange("p (k t) -> p k t", t=T)          # [128, 6, 512] = x^T
    w_v = w_d.rearrange("p (k e) -> p k e", e=D_E)        # [128, 6, 256] = W^T
    e_v = e_d.rearrange("p (j d) -> p j d", d=D_E)        # [128, 98, 256]
    o_v = o_d.rearrange("p (tt e) -> p tt e", e=E_PAD)    # [128, 4, 12544]

    # entity groups of 4 tiles (512 cols), tail group of 2
    groups = []
    t0 = 0
    while t0 < E_TILES:
        n = min(4, E_TILES - t0)
        groups.append((t0, n))
        t0 += n
    # pairs of groups -> 1024-wide output stores (last pair may be short)
    pairs = []
    gi = 0
    while gi < len(groups):
        pairs.append(groups[gi:gi + 2])
        gi += 2

    with tile.TileContext(nc) as tc, ExitStack() as ctx:
        const = ctx.enter_context(tc.tile_pool(name="const", bufs=1))
        psumA = ctx.enter_context(tc.tile_pool(name="psumA", bufs=2, space="PSUM"))
        psumB = ctx.enter_context(tc.tile_pool(name="psumB", bufs=2, space="PSUM"))

        # ---------------- constants ----------------
        identity_f = const.tile([128, 128], F32)
        make_identity(nc, identity_f)
        ident = const.tile([128, 128], BF16)
        nc.vector.tensor_copy(ident, identity_f)
        ones_f = const.tile([1, 128], F32)
        nc.vector.memset(ones_f, 1.0)
        ones_row = const.tile([1, 128], BF16)
        nc.vector.tensor_copy(ones_row, ones_f)
        eps_col = const.tile([128, 1], F32)
        nc.vector.memset(eps_col, EPS2)
        b_sb = const.tile([1, D_E], BF16)
        nc.sync.dma_start(out=b_sb, in_=b_d)
        qnt = const.tile([128, 2, T], BF16)   # q normalized+transposed [d, h, t]

        ent_pool = ctx.enter_context(tc.tile_pool(name="ent_pool", bufs=7))
        enT_pool = ctx.enter_context(tc.tile_pool(name="enT_pool", bufs=12))
        sq_pool = ctx.enter_context(tc.tile_pool(name="sq_pool", bufs=6))
        small = ctx.enter_context(tc.tile_pool(name="small", bufs=16))
        d_pool = ctx.enter_context(tc.tile_pool(name="d_pool", bufs=13))
        out_pool = ctx.enter_context(tc.tile_pool(name="out_pool", bufs=3))

        # x/W loads first and on the SAME ring as ent chunks (strict FIFO):
        # they gate q-setup -> first GEMM and must land before the 6.4MB
        # entity stream monopolizes the SDMA engines
        xt_g = const.tile([128, 6, T], BF16)     # [d_in_chunk, k, t]
        wt_g = const.tile([128, 6, D_E], BF16)   # [d_in_chunk, k, d_e]
        nc.sync.dma_start(out=xt_g, in_=x_v)
        nc.sync.dma_start(out=wt_g, in_=w_v)

        # ---------------- entity loads: big contiguous DMAs, issued upfront --
        chunk_tiles = []        # (j_start, n_tiles, tile)
        j0 = 0
        while j0 < E_TILES:
            n = min(CHUNK, E_TILES - j0)
            ct = ent_pool.tile([128, n, D_E], BF16, tag="ent", name=f"ent{j0}")
            nc.sync.dma_start(out=ct, in_=e_v[:, j0:j0 + n, :])
            chunk_tiles.append((j0, n, ct))
            j0 += n

        def ent_slice(j_start, n):
            ci = j_start // CHUNK
            c0, cn, ct = chunk_tiles[ci]
            lo = j_start - c0
            assert lo + n <= cn
            return ct[:, lo:lo + n, :]

        # ---------------- per-pair stage 1: norms + scaled transpose --------
        def stage1(pair, pi):
            jtot = sum(ng for _, ng in pair)
            ent_p = ent_slice(pair[0][0], jtot)   # pair is j-contiguous
            sq = sq_pool.tile([128, 8, D_E], BF16, tag="sq", name="sq")
            nc.vector.tensor_mul(sq[:, 0:jtot, :], ent_p, ent_p)
            nrm = small.tile([128, 8], F32, tag="nrm", name="nrm")
            nc.vector.reduce_sum(
                nrm[:, 0:jtot], sq[:, 0:jtot, :], mybir.AxisListType.X)
            s = small.tile([128, 8], F32, tag="s", name="s")
            nc.scalar.activation(s[:, 0:jtot], nrm[:, 0:jtot], AF.Sqrt,
                                 bias=eps_col)
            c_f = small.tile([128, 8], F32, tag="c", name="c")
            nc.vector.reciprocal(c_f[:, 0:jtot], s[:, 0:jtot])
            dmat = d_pool.tile([128, 8, 128], BF16, tag="D", name="D")
            nc.vector.tensor_mul(
                dmat[:, 0:jtot, :],
                identity_f[:, None, :].broadcast_to([128, jtot, 128]),
                c_f[:, 0:jtot, None].broadcast_to([128, jtot, 128]),
            )
            enTs = []
            joff = 0
            for k, (g0, ng) in enumerate(pair):
                ent = ent_slice(g0, ng)
                pT = psumA.tile([128, 4, 2, 128], F32, tag="pT", name="pT")
                for jj in range(ng):
                    for h in range(2):
                        nc.tensor.matmul(
                            pT[:, jj, h, :],
                            ent[:, jj, 128 * h:128 * (h + 1)],
                            dmat[:, joff + jj, :],
                            start=True, stop=True,
                        )
                enT = enT_pool.tile([128, 2, 512], BF16, tag="enT", name="enT")
                cp = (nc.vector.tensor_copy if (k == 1 and pi >= 5)
                      else nc.scalar.copy)
                cp(
                    enT.rearrange("p h (j e) -> p h j e", e=128)[:, :, 0:ng, :],
                    pT.rearrange("p j h e -> p h j e")[:, :, 0:ng, :],
                )
                enTs.append(enT)
                joff += ng
            return enTs

        # ---------------- per-pair stage 2: main GEMM + store ---------------
        def stage2(pair, enTs, pi):
            pw = sum(ng for _, ng in pair) * 128
            g0 = pair[0][0]
            ob = out_pool.tile([128, 4, 1024], BF16, tag="ob", name="ob")
            for pr in range(2):
                off = 0
                for (gg, ng), enT in zip(pair, enTs):
                    width = ng * 128
                    po = psumB.tile([128, 2, 512], F32, tag="po", name="po")
                    for i in range(2):
                        tt = 2 * pr + i
                        for h in range(2):
                            nc.tensor.matmul(
                                po[:, i, 0:width],
                                qnt[:, h, 128 * tt:128 * (tt + 1)],
                                enT[:, h, 0:width],
                                start=(h == 0),
                                stop=(h == 1),
                            )
                    cp = (nc.vector.tensor_copy
                          if (pr == 1 and gg == pair[-1][0] and pi >= 4)
                          else nc.scalar.copy)
                    cp(ob[:, 2 * pr:2 * pr + 2, off:off + width],
                       po[:, :, 0:width])
                    off += width
            if pi >= 11 and len(pair) == 2:
                w0 = pair[0][1] * 128
                nc.sync.dma_start(
                    out=o_v[:, :, g0 * 128:g0 * 128 + w0], in_=ob[:, :, 0:w0])
                nc.sync.dma_start(
                    out=o_v[:, :, g0 * 128 + w0:g0 * 128 + pw],
                    in_=ob[:, :, w0:pw])
            else:
                nc.sync.dma_start(
                    out=o_v[:, :, g0 * 128:g0 * 128 + pw], in_=ob[:, :, 0:pw])

        # ---------------- setup: load x/W, q = tanh(xW^T+b), qnt ------------
        setup_ctx = ExitStack()
        setup = setup_ctx.enter_context(tc.tile_pool(name="setup", bufs=1))
        xt = xt_g
        wt = wt_g

        q_sb = setup.tile([128, 4, D_E], BF16)
        sqq = setup.tile([128, D_E], F32)
        nrm_q = setup.tile([128, 4], F32)
        s_q = setup.tile([128, 4], F32)
        a_bf = setup.tile([128, 4], F32)
        for tt in range(4):
            psq = psumB.tile([128, 2, 512], F32, tag="po")
            pq = psq.rearrange("p a b -> p (a b)")
            for k in range(6):
                nc.tensor.matmul(
                    pq[:, 0:D_E],
                    xt[:, k, 128 * tt:128 * (tt + 1)],
                    wt[:, k, :],
                    start=(k == 0),
                    stop=False,
                )
            nc.tensor.matmul(pq[:, 0:D_E], ones_row, b_sb,
                             start=False, stop=True)
            nc.scalar.activation(q_sb[:, tt, :], pq[:, 0:D_E], AF.Tanh)
            # squares + row-sum fused on scalar (keeps the q chain off the
            # vector engine, whose queue fills with run-ahead stage1 work)
            nc.scalar.activation(sqq, q_sb[:, tt, :], AF.Square,
                                 accum_out=nrm_q[:, tt:tt + 1])
        nc.scalar.activation(s_q, nrm_q, AF.Sqrt, bias=eps_col)
        nc.vector.reciprocal(a_bf, s_q)
        d_a = setup.tile([128, 4, 128], BF16)
        for tt in range(4):
            nc.scalar.activation(d_a[:, tt, :], ident, AF.Copy,
                                 scale=a_bf[:, tt:tt + 1])
        ps_q = psumA.tile([128, 4, 2, 128], F32, tag="pT")
        pq_v = ps_q.rearrange("p a b c -> p (a b) c")  # [128, 8, 128]
        for h in range(2):
            for tt in range(4):
                nc.tensor.matmul(
                    pq_v[:, 4 * h + tt, :],
                    q_sb[:, tt, 128 * h:128 * (h + 1)],
                    d_a[:, tt, :],
                    start=True, stop=True,
                )
        nc.scalar.copy(
            qnt.rearrange("p h (tt t) -> p h tt t", t=128),
            pq_v.rearrange("p (h tt) t -> p h tt t", h=2),
        )
        setup_ctx.close()  # release setup SBUF

        # ---------------- prologue: stage1 for first SKEW pairs -------------
        SKEW = 2
        PRO = min(SKEW, len(pairs))
        enTs_map = {}
        for pi in range(PRO):
            enTs_map[pi] = stage1(pairs[pi], pi)

        # ---------------- steady state ----------------
        for di in range(len(pairs)):
            stage2(pairs[di], enTs_map.pop(di), di)
            pi = di + PRO
            if pi < len(pairs):
                enTs_map[pi] = stage1(pairs[pi], pi)

    nc.compile()
    return nc


_CACHE = {}


def _best_effort_device_reset():
    """Recover wedged NeuronCores if the axon PJRT library is present."""
    try:
        import ctypes

        if os.path.exists("/opt/axon/libaxon_pjrt.so"):
            lib = ctypes.CDLL("/opt/axon/libaxon_pjrt.so")
            if hasattr(lib, "axon_reset"):
                lib.axon_reset.restype = ctypes.c_int64
                lib.axon_reset()
    except Exception:
        pass


def _get_nc():
    if "nc" not in _CACHE:
        _best_effort_device_reset()
        _CACHE["nc"] = build_nc()
    return _CACHE["nc"]


def kernel(x, W, b, entity_embedding, trace=False):
    from concourse.bass_utils import run_bass_kernel_spmd

    nc = _get_nc()

    # x: [4,128,768] -> x^T p-major [128, 6, 512] (t = tt*128 + p)
    x2 = np.asarray(x, dtype=np.float32).reshape(T, D_IN)
    xt = np.ascontiguousarray(
        x2.T.reshape(6, 128, T).transpose(1, 0, 2)
    ).astype(bfloat16).reshape(128, 6 * T)
    # W: [256,768] -> W^T p-major [128, 6, 256]
    w2 = np.asarray(W, dtype=np.float32)
    wt = np.ascontiguousarray(
        w2.T.reshape(6, 128, D_E).transpose(1, 0, 2)
    ).astype(bfloat16).reshape(128, 6 * D_E)
    b2 = np.asarray(b, dtype=np.float32).reshape(1, D_E).astype(bfloat16)
    ent = np.asarray(entity_embedding, dtype=np.float32)

    pad = np.ones((E_PAD - E_PER_CORE, D_E), dtype=np.float32)
    in_maps = []
    for i in range(N_CORES):
        shard = np.concatenate(
            [ent[i * E_PER_CORE:(i + 1) * E_PER_CORE], pad], axis=0
        ).astype(bfloat16)
        shard = np.ascontiguousarray(
            shard.reshape(E_TILES, 128, D_E).transpose(1, 0, 2)
        ).reshape(128, E_TILES * D_E)
        in_maps.append({"x": xt, "w": wt, "b": b2, "ent": shard})

    res = run_bass_kernel_spmd(nc, in_maps, core_ids=list(range(N_CORES)),
                               trace=trace)
    kernel.last = res
    outs = []
    for i in range(N_CORES):
        o = np.asarray(res.results[i]["out"]).reshape(128, 4, E_PAD)
        o = o.transpose(1, 0, 2).reshape(T, E_PAD)[:, :E_PER_CORE]
        outs.append(o)
    full = np.concatenate(outs, axis=1).astype(np.float32)
    return np.ascontiguousarray(full.reshape(4, 128, E_FULL))


kernel.last = None


# revision 34
# speedup vs baseline: 1.0128x; 1.0128x over previous
"""EntityDisambiguationHead Trainium2 kernel (bf16 rewrite).

Computes out[b,s,e] = cos_sim(tanh(x @ W.T + b), entity_embedding[e]) for
B=4, S=128, D_in=768, D_e=256, E=100000, sharding the entity axis across
8 NeuronCores (each core handles 12544 = 98*128 entities, padded from 12500).

All wire traffic is bf16 (host casts + relayouts); GEMMs run bf16 on the PE
with f32 PSUM accumulation. Entity L2-normalization is fused into the PE
transpose by multiplying with diag(1/||e||) instead of the identity:

  per core:
    q   = tanh(x @ W.T + b)               [512, 256]   (PE, bf16)
    a   = 1/sqrt(||q_t||^2 + eps)         per token
    qnt = (q^T scaled by a)               via PE matmul vs diag(a)
    c   = 1/sqrt(||ent_e||^2 + eps)       per entity (sq+reduce+rsqrt)
    enT = ent_tile^T @ diag(c)            transposed + normalized on PE
    out = qnt^T @ enT -> bf16             [512, 12544]

Host side only shards/casts/relayouts inputs and concatenates outputs.
DMA: entity loads stream on the scalar HWDGE ring (7 big contiguous
transfers issued upfront), stores go out on the sync ring.
"""

import os
from contextlib import ExitStack

import numpy as np
from ml_dtypes import bfloat16

import concourse.bass as bass
import concourse.bacc as bacc
import concourse.mybir as mybir
import concourse.tile as tile
from concourse.masks import make_identity

F32 = mybir.dt.float32
BF16 = mybir.dt.bfloat16
AF = mybir.ActivationFunctionType
ALU = mybir.AluOpType

N_CORES = 8
E_FULL = 100000
E_PER_CORE = E_FULL // N_CORES          # 12500
E_TILES = (E_PER_CORE + 127) // 128     # 98
E_PAD = E_TILES * 128                   # 12544
T = 512                                 # tokens = 4*128
D_IN = 768
D_E = 256
EPS2 = 1e-16                            # added under sqrt ~= max(norm, 1e-8)

CHUNK = 16                              # entity tiles per load DMA


def build_nc():
    """Build the per-core Bass program (SPMD: same program on all cores)."""
    nc = bacc.Bacc("TRN2", target_bir_lowering=False, debug=False)

    # DRAM tensors in device-friendly (partition-major) layouts; host prepares.
    x_d = nc.dram_tensor("x", [128, 6 * T], BF16, kind="ExternalInput").ap()
    w_d = nc.dram_tensor("w", [128, 6 * D_E], BF16, kind="ExternalInput").ap()
    b_d = nc.dram_tensor("b", [1, D_E], BF16, kind="ExternalInput").ap()
    e_d = nc.dram_tensor("ent", [128, E_TILES * D_E], BF16, kind="ExternalInput").ap()
    o_d = nc.dram_tensor("out", [128, 4 * E_PAD], BF16, kind="ExternalOutput").ap()

    x_v = x_d.rearrange("p (k t) -> p k t", t=T)          # [128, 6, 512] = x^T
    w_v = w_d.rearrange("p (k e) -> p k e", e=D_E)        # [128, 6, 256] = W^T
    e_v = e_d.rearrange("p (j d) -> p j d", d=D_E)        # [128, 98, 256]
    o_v = o_d.rearrange("p (tt e) -> p tt e", e=E_PAD)    # [128, 4, 12544]

    # entity groups of 4 tiles (512 cols), tail group of 2
    groups = []
    t0 = 0
    while t0 < E_TILES:
        n = min(4, E_TILES - t0)
        groups.append((t0, n))
        t0 += n
    # pairs of groups -> 1024-wide output stores (last pair may be short)
    pairs = []
    gi = 0
    while gi < len(groups):
        pairs.append(groups[gi:gi + 2])
        gi += 2

    with tile.TileContext(nc) as tc, ExitStack() as ctx:
        const = ctx.enter_context(tc.tile_pool(name="const", bufs=1))
        psumA = ctx.enter_context(tc.tile_pool(name="psumA", bufs=2, space="PSUM"))
        psumB = ctx.enter_context(tc.tile_pool(name="psumB", bufs=2, space="PSUM"))

        # ---------------- constants ----------------
        identity_f = const.tile([128, 128], F32)
        make_identity(nc, identity_f)
        ident = const.tile([128, 128], BF16)
        nc.vector.tensor_copy(ident, identity_f)
        ones_f = const.tile([1, 128], F32)
        nc.vector.memset(ones_f, 1.0)
        ones_row = const.tile([1, 128], BF16)
        nc.vector.tensor_copy(ones_row, ones_f)
        eps_col = const.tile([128, 1], F32)
        nc.vector.memset(eps_col, EPS2)
        b_sb = const.tile([1, D_E], BF16)
        nc.sync.dma_start(out=b_sb, in_=b_d)
        qnt = const.tile([128, 2, T], BF16)   # q normalized+transposed [d, h, t]

        ent_pool = ctx.enter_context(tc.tile_pool(name="ent_pool", bufs=7))
        enT_pool = ctx.enter_context(tc.tile_pool(name="enT_pool", bufs=12))
        sq_pool = ctx.enter_context(tc.tile_pool(name="sq_pool", bufs=6))
        small = ctx.enter_context(tc.tile_pool(name="small", bufs=16))
        d_pool = ctx.enter_context(tc.tile_pool(name="d_pool", bufs=13))
        out_pool = ctx.enter_context(tc.tile_pool(name="out_pool", bufs=3))

        # x/W loads first and on the SAME ring as ent chunks (strict FIFO):
        # they gate q-setup -> first GEMM and must land before the 6.4MB
        # entity stream monopolizes the SDMA engines
        xt_g = const.tile([128, 6, T], BF16)     # [d_in_chunk, k, t]
        wt_g = const.tile([128, 6, D_E], BF16)   # [d_in_chunk, k, d_e]
        nc.sync.dma_start(out=xt_g, in_=x_v)
        nc.sync.dma_start(out=wt_g, in_=w_v)

        # ---------------- entity loads: big contiguous DMAs, issued upfront --
        chunk_tiles = []        # (j_start, n_tiles, tile)
        j0 = 0
        while j0 < E_TILES:
            n = min(CHUNK, E_TILES - j0)
            ct = ent_pool.tile([128, n, D_E], BF16, tag="ent", name=f"ent{j0}")
            nc.sync.dma_start(out=ct, in_=e_v[:, j0:j0 + n, :])
            chunk_tiles.append((j0, n, ct))
            j0 += n

        def ent_slice(j_start, n):
            ci = j_start // CHUNK
            c0, cn, ct = chunk_tiles[ci]
            lo = j_start - c0
            assert lo + n <= cn
            return ct[:, lo:lo + n, :]

        # ---------------- per-pair stage 1: norms + scaled transpose --------
        def stage1(pair, pi):
            jtot = sum(ng for _, ng in pair)
            ent_p = ent_slice(pair[0][0], jtot)   # pair is j-contiguous
            sq = sq_pool.tile([128, 8, D_E], BF16, tag="sq", name="sq")
            nc.vector.tensor_mul(sq[:, 0:jtot, :], ent_p, ent_p)
            nrm = small.tile([128, 8], F32, tag="nrm", name="nrm")
            nc.vector.reduce_sum(
                nrm[:, 0:jtot], sq[:, 0:jtot, :], mybir.AxisListType.X)
            s = small.tile([128, 8], F32, tag="s", name="s")
            nc.scalar.activation(s[:, 0:jtot], nrm[:, 0:jtot], AF.Sqrt,
                                 bias=eps_col)
            c_f = small.tile([128, 8], F32, tag="c", name="c")
            nc.vector.reciprocal(c_f[:, 0:jtot], s[:, 0:jtot])
            dmat = d_pool.tile([128, 8, 128], BF16, tag="D", name="D")
            nc.vector.tensor_mul(
                dmat[:, 0:jtot, :],
                identity_f[:, None, :].broadcast_to([128, jtot, 128]),
                c_f[:, 0:jtot, None].broadcast_to([128, jtot, 128]),
            )
            enTs = []
            joff = 0
            for k, (g0, ng) in enumerate(pair):
                ent = ent_slice(g0, ng)
                pT = psumA.tile([128, 4, 2, 128], F32, tag="pT", name="pT")
                for jj in range(ng):
                    for h in range(2):
                        nc.tensor.matmul(
                            pT[:, jj, h, :],
                            ent[:, jj, 128 * h:128 * (h + 1)],
                            dmat[:, joff + jj, :],
                            start=True, stop=True,
                        )
                enT = enT_pool.tile([128, 2, 512], BF16, tag="enT", name="enT")
                cp = (nc.vector.tensor_copy if (k == 1 and pi >= 6)
                      else nc.scalar.copy)
                cp(
                    enT.rearrange("p h (j e) -> p h j e", e=128)[:, :, 0:ng, :],
                    pT.rearrange("p j h e -> p h j e")[:, :, 0:ng, :],
                )
                enTs.append(enT)
                joff += ng
            return enTs

        # ---------------- per-pair stage 2: main GEMM + store ---------------
        def stage2(pair, enTs, pi):
            pw = sum(ng for _, ng in pair) * 128
            g0 = pair[0][0]
            ob = out_pool.tile([128, 4, 1024], BF16, tag="ob", name="ob")
            for pr in range(2):
                off = 0
                for (gg, ng), enT in zip(pair, enTs):
                    width = ng * 128
                    po = psumB.tile([128, 2, 512], F32, tag="po", name="po")
                    for i in range(2):
                        tt = 2 * pr + i
                        for h in range(2):
                            nc.tensor.matmul(
                                po[:, i, 0:width],
                                qnt[:, h, 128 * tt:128 * (tt + 1)],
                                enT[:, h, 0:width],
                                start=(h == 0),
                                stop=(h == 1),
                            )
                    cp = (nc.vector.tensor_copy
                          if (pr == 1 and gg == pair[-1][0] and pi >= 5)
                          else nc.scalar.copy)
                    cp(ob[:, 2 * pr:2 * pr + 2, off:off + width],
                       po[:, :, 0:width])
                    off += width
            if pi >= 11 and len(pair) == 2:
                w0 = pair[0][1] * 128
                nc.sync.dma_start(
                    out=o_v[:, :, g0 * 128:g0 * 128 + w0], in_=ob[:, :, 0:w0])
                nc.sync.dma_start(
                    out=o_v[:, :, g0 * 128 + w0:g0 * 128 + pw],
                    in_=ob[:, :, w0:pw])
            else:
                nc.sync.dma_start(
                    out=o_v[:, :, g0 * 128:g0 * 128 + pw], in_=ob[:, :, 0:pw])

        # ---------------- setup: load x/W, q = tanh(xW^T+b), qnt ------------
        setup_ctx = ExitStack()
        setup = setup_ctx.enter_context(tc.tile_pool(name="setup", bufs=1))
        xt = xt_g
        wt = wt_g

        q_sb = setup.tile([128, 4, D_E], BF16)
        sqq = setup.tile([128, D_E], F32)
        nrm_q = setup.tile([128, 4], F32)
        s_q = setup.tile([128, 4], F32)
        a_bf = setup.tile([128, 4], F32)
        for tt in range(4):
            psq = psumB.tile([128, 2, 512], F32, tag="po")
            pq = psq.rearrange("p a b -> p (a b)")
            for k in range(6):
                nc.tensor.matmul(
                    pq[:, 0:D_E],
                    xt[:, k, 128 * tt:128 * (tt + 1)],
                    wt[:, k, :],
                    start=(k == 0),
                    stop=False,
                )
            nc.tensor.matmul(pq[:, 0:D_E], ones_row, b_sb,
                             start=False, stop=True)
            nc.scalar.activation(q_sb[:, tt, :], pq[:, 0:D_E], AF.Tanh)
            # squares + row-sum fused on scalar (keeps the q chain off the
            # vector engine, whose queue fills with run-ahead stage1 work)
            nc.scalar.activation(sqq, q_sb[:, tt, :], AF.Square,
                                 accum_out=nrm_q[:, tt:tt + 1])
        nc.scalar.activation(s_q, nrm_q, AF.Sqrt, bias=eps_col)
        nc.vector.reciprocal(a_bf, s_q)
        d_a = setup.tile([128, 4, 128], BF16)
        for tt in range(4):
            nc.scalar.activation(d_a[:, tt, :], ident, AF.Copy,
                                 scale=a_bf[:, tt:tt + 1])
        ps_q = psumA.tile([128, 4, 2, 128], F32, tag="pT")
        pq_v = ps_q.rearrange("p a b c -> p (a b) c")  # [128, 8, 128]
        for h in range(2):
            for tt in range(4):
                nc.tensor.matmul(
                    pq_v[:, 4 * h + tt, :],
                    q_sb[:, tt, 128 * h:128 * (h + 1)],
                    d_a[:, tt, :],
                    start=True, stop=True,
                )
        nc.scalar.copy(
            qnt.rearrange("p h (tt t) -> p h tt t", t=128),
            pq_v.rearrange("p (h tt) t -> p h tt t", h=2),
        )
        setup_ctx.close()  # release setup SBUF

        # ---------------- prologue: stage1 for first SKEW pairs -------------
        SKEW = 2
        PRO = min(SKEW, len(pairs))
        enTs_map = {}
        for pi in range(PRO):
            enTs_map[pi] = stage1(pairs[pi], pi)

        # ---------------- steady state ----------------
        for di in range(len(pairs)):
            stage2(pairs[di], enTs_map.pop(di), di)
            pi = di + PRO
            if pi < len(pairs):
                enTs_map[pi] = stage1(pairs[pi], pi)

    nc.compile()
    return nc


_CACHE = {}


def _best_effort_device_reset():
    """Recover wedged NeuronCores if the axon PJRT library is present."""
    try:
        import ctypes

        if os.path.exists("/opt/axon/libaxon_pjrt.so"):
            lib = ctypes.CDLL("/opt/axon/libaxon_pjrt.so")
            if hasattr(lib, "axon_reset"):
                lib.axon_reset.restype = ctypes.c_int64
                lib.axon_reset()
    except Exception:
        pass


def _get_nc():
    if "nc" not in _CACHE:
        _best_effort_device_reset()
        _CACHE["nc"] = build_nc()
    return _CACHE["nc"]


def kernel(x, W, b, entity_embedding, trace=False):
    from concourse.bass_utils import run_bass_kernel_spmd

    nc = _get_nc()

    # x: [4,128,768] -> x^T p-major [128, 6, 512] (t = tt*128 + p)
    x2 = np.asarray(x, dtype=np.float32).reshape(T, D_IN)
    xt = np.ascontiguousarray(
        x2.T.reshape(6, 128, T).transpose(1, 0, 2)
    ).astype(bfloat16).reshape(128, 6 * T)
    # W: [256,768] -> W^T p-major [128, 6, 256]
    w2 = np.asarray(W, dtype=np.float32)
    wt = np.ascontiguousarray(
        w2.T.reshape(6, 128, D_E).transpose(1, 0, 2)
    ).astype(bfloat16).reshape(128, 6 * D_E)
    b2 = np.asarray(b, dtype=np.float32).reshape(1, D_E).astype(bfloat16)
    ent = np.asarray(entity_embedding, dtype=np.float32)

    pad = np.ones((E_PAD - E_PER_CORE, D_E), dtype=np.float32)
    in_maps = []
    for i in range(N_CORES):
        shard = np.concatenate(
            [ent[i * E_PER_CORE:(i + 1) * E_PER_CORE], pad], axis=0
        ).astype(bfloat16)
        shard = np.ascontiguousarray(
            shard.reshape(E_TILES, 128, D_E).transpose(1, 0, 2)
        ).reshape(128, E_TILES * D_E)
        in_maps.append({"x": xt, "w": wt, "b": b2, "ent": shard})

    res = run_bass_kernel_spmd(nc, in_maps, core_ids=list(range(N_CORES)),
                               trace=trace)
    kernel.last = res
    outs = []
    for i in range(N_CORES):
        o = np.asarray(res.results[i]["out"]).reshape(128, 4, E_PAD)
        o = o.transpose(1, 0, 2).reshape(T, E_PAD)[:, :E_PER_CORE]
        outs.append(o)
    full = np.concatenate(outs, axis=1).astype(np.float32)
    return np.ascontiguousarray(full.reshape(4, 128, E_FULL))


kernel.last = None
